# revision 25
# baseline (speedup 1.0000x reference)
"""DIIN Trainium2 Bass kernel.

Data-parallel over batch (B=8 -> 8 cores, one batch element per core).
Per core: self-attention encode of p/h (fp32), interaction tensor,
1x1 conv-in, 3 DenseNet blocks (8 conv3x3 layers each, growth 20) with
1x1-conv + 2x2-maxpool transitions, final FC -> 2 logits (fp32).

Conv3x3 mapping: weight-stationary matmuls with the 9 kernel offsets as
separate PSUM-accumulated matmuls; the shifted input windows are plain
strided access patterns over zero-padded [C, H+2, W+2] SBUF feature maps
(no im2col materialization). Output channels (20) use only a sliver of
the PE array column dim, so each spatial group is split into four chunks
packed onto the four 32-column groups of the PE array (tile_position via
the psum base partition); the four streams execute concurrently on
disjoint 32x32 sub-array columns. Each column group accumulates in its
own PSUM bank (a single accumulation group must not span column groups:
start=True clears has_written for the whole bank).

The conv path runs in bf16 (fp32 PSUM accumulation): fp32 matmuls stream
at 4 cycles/row on TRN2 vs 1 for bf16, and fp32r (1 cycle/row at N>=256)
requires dst partition 0, which defeats the column packing. Set
CONV_DT=f32 for a bit-accurate (~4e-7 rel err) but ~3-4x slower variant;
bf16 measures ~2e-3 rel err end-to-end.

Epilogue: per group, 4 relu ops (ScalarE/VectorE alternating) compact the
per-column-group PSUM slices to partition base 0 of a [32, 4, N] staging
tile (engine SBUF accesses must start at 32-aligned partitions), then one
SBUF->SBUF DMA scatters the 20 new channels into the padded feature map
at their (arbitrary, DMA-only) partition rows - the in-place DenseNet
concat.
"""

import os
import sys
import numpy as np

if "/opt/trn_rl_repo" not in sys.path:
    sys.path.insert(0, "/opt/trn_rl_repo")

CONV_DT = os.environ.get("CONV_DT", "bf16")  # "bf16" or "f32"

# ----------------------------------------------------------------- constants
B, P, E, D = 8, 128, 768, 128
GR, NL = 20, 8
C0 = 38

# per block: spatial W(=H), X-tile channel capacities, input channels of layer0,
# psum spatial groups, rows per col-group chunk
BLK = [
    dict(W=128, caps=[128, 70], cin0=38, groups=8, rpc=4),
    dict(W=64, caps=[128, 128, 3], cin0=99, groups=2, rpc=8),
    dict(W=32, caps=[128, 128, 33], cin0=129, groups=1, rpc=8),
]
# transitions: input channels, output M-tiles [(chan_lo, n)], input spatial W
TRS = [
    dict(cin=198, mts=[(0, 99)], W=128),
    dict(cin=259, mts=[(0, 128), (128, 1)], W=64),
    dict(cin=289, mts=[(0, 128), (128, 16)], W=32),
]


def _chunks_of(cin, caps):
    """contraction chunks [(tile_idx, kc)] covering channels [0, cin)"""
    out = []
    for ti, cap in enumerate(caps):
        base = 128 * ti
        if base >= cin:
            break
        out.append((ti, min(cin - base, cap)))
    return out


def _pieces_of(lo, hi, caps):
    """[(tile_idx, local_lo, off_in_range, n)] covering channels [lo, hi)"""
    out = []
    for ti, cap in enumerate(caps):
        base = 128 * ti
        a, b = max(lo, base), min(hi, base + cap)
        if a < b:
            out.append((ti, a - base, a - lo, b - a))
    return out


def _dense_cols(bl):
    """column offsets for the packed dense-conv weight tensor of block bl"""
    cols = {}
    c = 0
    spec = BLK[bl]
    for l in range(NL):
        cin = spec["cin0"] + GR * l
        for ci, (ti, kc) in enumerate(_chunks_of(cin, spec["caps"])):
            for ky in (-1, 0, 1):
                for kx in (-1, 0, 1):
                    cols[(l, ci, ky, kx)] = c
                    c += GR
    return cols, c


def _trans_cols(bl):
    cols = {}
    c = 0
    spec = TRS[bl]
    for ci, (ti, kc) in enumerate(_chunks_of(spec["cin"], BLK[bl]["caps"])):
        for mi, (mlo, mn) in enumerate(spec["mts"]):
            cols[(ci, mi)] = c
            c += mn
    return cols, c


DCOLS = [_dense_cols(b) for b in range(3)]
TCOLS = [_trans_cols(b) for b in range(3)]


# ----------------------------------------------------------------- bass build
def _build_nc():
    import os
    PH = int(os.environ.get("KERNEL_PHASES", "9"))
    import concourse.bacc as bacc
    import concourse.tile as tile
    import concourse.mybir as mybir
    from concourse.masks import make_identity

    f32 = mybir.dt.float32
    cdt = mybir.dt.bfloat16 if CONV_DT == "bf16" else mybir.dt.float32
    AF = mybir.ActivationFunctionType
    ALU = mybir.AluOpType
    AX = mybir.AxisListType

    nc = bacc.Bacc(None, target_bir_lowering=False)

    # ---- dram I/O
    pT_d = nc.dram_tensor("pT", [E, P], f32, kind="ExternalInput")
    hT_d = nc.dram_tensor("hT", [E, P], f32, kind="ExternalInput")
    WtT_d = nc.dram_tensor("WtT", [E, D], f32, kind="ExternalInput")
    bt_d = nc.dram_tensor("bt", [D, 1], f32, kind="ExternalInput")
    wup_d = nc.dram_tensor("wup", [D, 1], f32, kind="ExternalInput")
    wprod_d = nc.dram_tensor("wprod", [D, 1], f32, kind="ExternalInput")
    WcT_d = nc.dram_tensor("WcT", [D, C0], cdt, kind="ExternalInput")
    biascol_d = nc.dram_tensor("biascol", [128, 1], f32, kind="ExternalInput")
    Wd_d = [
        nc.dram_tensor(f"Wd{i}", [128, DCOLS[i][1]], cdt, kind="ExternalInput")
        for i in range(3)
    ]
    Tw_d = [
        nc.dram_tensor(f"Tw{i}", [128, TCOLS[i][1]], cdt, kind="ExternalInput")
        for i in range(3)
    ]
    Wfc_d = nc.dram_tensor("Wfc", [128, 2, 2, 256], f32, kind="ExternalInput")
    bfc_d = nc.dram_tensor("bfc", [1, 2], f32, kind="ExternalInput")
    zeros_d = nc.dram_tensor("zeros", [128, 132], cdt, kind="ExternalInput")
    out_d = nc.dram_tensor("out", [1, 2], f32, kind="ExternalOutput")

    with tile.TileContext(nc) as tc:
        from contextlib import ExitStack

        with ExitStack() as ctx:
            persist = ctx.enter_context(tc.tile_pool(name="persist", bufs=1))
            # persistent across block boundaries
            X2a = persist.tile([128, 66, 66], cdt, tag="X2a")
            X3a = persist.tile([128, 34, 34], cdt, tag="X3a")
            X3b = persist.tile([128, 34, 34], cdt, tag="X3b")

            def memset_borders(t, w):
                n = t.shape[0]
                nc.sync.dma_start(out=t[:, 0, :], in_=zeros_d[0:n, 0 : w + 2])
                nc.sync.dma_start(out=t[:, w + 1, :], in_=zeros_d[0:n, 0 : w + 2])
                nc.sync.dma_start(out=t[:, :, 0], in_=zeros_d[0:n, 0 : w + 2])
                nc.sync.dma_start(out=t[:, :, w + 1], in_=zeros_d[0:n, 0 : w + 2])

            # ====================================================== phase b1
            with ExitStack() as c1:
                p_b1 = c1.enter_context(tc.tile_pool(name="p_b1", bufs=1))
                Wsb1 = p_b1.tile([128, DCOLS[0][1]], cdt, tag="Wsb1")
                nc.sync.dma_start(out=Wsb1[:], in_=Wd_d[0][:])
                T1sb = p_b1.tile([128, TCOLS[0][1]], cdt, tag="T1sb")
                nc.sync.dma_start(out=T1sb[:], in_=Tw_d[0][:])
                WcT_sb = p_b1.tile([D, C0], cdt, tag="WcT")
                nc.sync.dma_start(out=WcT_sb[:], in_=WcT_d[:])
                biascol = p_b1.tile([128, 1], f32, tag="biascol")
                nc.sync.dma_start(out=biascol[:], in_=biascol_d[:])
                X1a = p_b1.tile([128, 130, 130], cdt, tag="X1a")
                peT = p_b1.tile([D, P], f32, tag="peT")
                heT = p_b1.tile([D, P], f32, tag="heT")
                memset_borders(X1a, 128)
                memset_borders(X2a, 64)
                memset_borders(X3a, 32)
                memset_borders(X3b, 32)

                # ---------------------------------------------- encoders
                with ExitStack() as ce:
                    enc = ce.enter_context(tc.tile_pool(name="enc", bufs=2))
                    encps = ce.enter_context(
                        tc.tile_pool(name="encps", bufs=2, space="PSUM")
                    )
                    consts = ce.enter_context(tc.tile_pool(name="consts", bufs=1))
                    ident = consts.tile([128, 128], f32, tag="ident")
                    make_identity(nc, ident[:])
                    WtT_sb = consts.tile([128, 6, 128], f32, tag="WtT")
                    nc.sync.dma_start(
                        out=WtT_sb[:], in_=WtT_d.rearrange("(k p) j -> p k j", p=128)
                    )
                    bt_sb = consts.tile([D, 1], f32, tag="bt")
                    nc.sync.dma_start(out=bt_sb[:], in_=bt_d[:])
                    wup_sb = consts.tile([D, 1], f32, tag="wup")
                    nc.sync.dma_start(out=wup_sb[:], in_=wup_d[:])
                    wprod_sb = consts.tile([D, 1], f32, tag="wprod")
                    nc.sync.dma_start(out=wprod_sb[:], in_=wprod_d[:])
                    ones1 = consts.tile([1, 128], f32, tag="ones1")
                    nc.vector.memset(ones1[:], 1.0)

                    for name, src_d, dst in (("p", pT_d, peT), ("h", hT_d, heT)):
                        with nc.named_scope(f"enc_{name}"):
                            XE = enc.tile([128, 6, 128], f32, tag="XE")
                            nc.sync.dma_start(
                                out=XE[:], in_=src_d.rearrange("(k p) j -> p k j", p=128)
                            )
                            psxT = encps.tile([128, 128], f32, tag="psxT")
                            for k in range(6):
                                nc.tensor.matmul(
                                    psxT[:],
                                    WtT_sb[:, k, :],
                                    XE[:, k, :],
                                    start=(k == 0),
                                    stop=(k == 5),
                                )
                            xT = enc.tile([128, 128], f32, tag="xT")
                            nc.scalar.activation(
                                xT[:], psxT[:], AF.Identity, bias=bt_sb[:]
                            )
                            # x (token-major) via PE transpose
                            psT = encps.tile([128, 128], f32, tag="psT")
                            nc.tensor.transpose(psT[:], xT[:], ident[:])
                            x_sb = enc.tile([128, 128], f32, tag="x_sb")
                            nc.vector.tensor_copy(x_sb[:], psT[:])
                            # u row
                            psu = encps.tile([1, 128], f32, tag="psu")
                            nc.tensor.matmul(
                                psu[0:1, :], wup_sb[:, 0:1], xT[:], start=True, stop=True
                            )
                            u_sb = enc.tile([1, 128], f32, tag="u_sb")
                            nc.vector.tensor_copy(u_sb[:], psu[0:1, :])
                            # A = (x*wprod) @ x.T + u[None, :]
                            xwT = enc.tile([128, 128], f32, tag="xwT")
                            nc.vector.tensor_scalar_mul(xwT[:], xT[:], wprod_sb[:, 0:1])
                            psA = encps.tile([128, 128], f32, tag="psA")
                            nc.tensor.matmul(psA[:], xwT[:], xT[:], start=True, stop=False)
                            nc.tensor.matmul(
                                psA[:], ones1[0:1, :], u_sb[0:1, :], start=False, stop=True
                            )
                            # softmax over free dim
                            negmax = enc.tile([128, 1], f32, tag="negmax")
                            nc.vector.tensor_reduce(
                                negmax[:], psA[:], axis=AX.X, op=ALU.max, negate=True
                            )
                            sAe = enc.tile([128, 128], f32, tag="sAe")
                            den = enc.tile([128, 1], f32, tag="den")
                            nc.scalar.activation(
                                sAe[:], psA[:], AF.Exp, bias=negmax[:], scale=1.0,
                                accum_out=den[:],
                            )
                            rden = enc.tile([128, 1], f32, tag="rden")
                            nc.vector.reciprocal(rden[:], den[:])
                            sAn = enc.tile([128, 128], f32, tag="sAn")
                            nc.vector.tensor_scalar_mul(sAn[:], sAe[:], rden[:])
                            psT2 = encps.tile([128, 128], f32, tag="psT")
                            nc.tensor.transpose(psT2[:], sAn[:], ident[:])
                            sAT = enc.tile([128, 128], f32, tag="sAT")
                            nc.vector.tensor_copy(sAT[:], psT2[:])
                            pspeT = encps.tile([128, 128], f32, tag="psA")
                            nc.tensor.matmul(
                                pspeT[:], x_sb[:], sAT[:], start=True, stop=True
                            )
                            nc.scalar.copy(dst[:], pspeT[:])

                # ------------------------------------- interaction + conv_in
                if PH >= 2:
                 with ExitStack() as ci:
                    pI = ci.enter_context(tc.tile_pool(name="pI", bufs=1))
                    cips = ci.enter_context(
                        tc.tile_pool(name="cips", bufs=2, space="PSUM")
                    )
                    stg0 = ci.enter_context(tc.tile_pool(name="stg0", bufs=3))
                    I = pI.tile([128, 128, 128], cdt, tag="I")  # [d, p, h]
                    with nc.named_scope("interaction"):
                        for p in range(P):
                            nc.vector.tensor_scalar_mul(
                                I[:, p, :], heT[:], peT[:, p : p + 1]
                            )
                    with nc.named_scope("conv_in"):
                        for g in range(16):
                            psl = [
                                cips.tile([128, 512], f32, tag=f"cip{cg}",
                                          name=f"cip{cg}")
                                for cg in range(2)
                            ]
                            for cg in range(2):
                                rhs = I[:, g * 8 + cg * 4 : g * 8 + cg * 4 + 4, :]
                                nc.tensor.matmul(
                                    psl[cg][64 * cg : 64 * cg + C0, :],
                                    WcT_sb[:],
                                    rhs,
                                    start=True,
                                    stop=True,
                                    tile_position=(0, 64 * cg),
                                )
                            st0 = stg0.tile([64, 2, 512], cdt, tag="st0")
                            for cg in range(2):
                                src = psl[cg][64 * cg : 64 * cg + C0, :]
                                dst = st0[0:C0, cg, :]
                                if cg == 0:
                                    nc.scalar.activation(
                                        dst, src, AF.Identity, bias=biascol[0:C0, :]
                                    )
                                else:
                                    nc.vector.tensor_scalar_add(
                                        dst, src, biascol[0:C0, 0:1]
                                    )
                            nc.sync.dma_start(
                                out=X1a[0:C0, g * 8 + 1 : g * 8 + 9, 1:129],
                                in_=st0[0:C0].rearrange(
                                    "c g (r x) -> c (g r) x", x=128
                                ),
                            )

                # ------------------------------------------- dense block 1
                if PH >= 3:
                 with ExitStack() as cb:
                    px1b = cb.enter_context(tc.tile_pool(name="px1b", bufs=1))
                    X1b = px1b.tile([70, 130, 130], cdt, tag="X1b")
                    memset_borders(X1b, 128)
                    with tc.tile_pool(name="dps1", bufs=2, space="PSUM") as dps, \
                            tc.tile_pool(name="stg1", bufs=3) as stg:
                        _dense_block(nc, tc, 0, [X1a, X1b], Wsb1, dps, stg)
                    # transition 1 -> X2a
                    if PH >= 4:
                     with tc.tile_pool(name="tps1", bufs=2, space="PSUM") as tps, \
                            tc.tile_pool(name="tmp1", bufs=2) as tmpp:
                        _transition(nc, tc, 0, [X1a, X1b], T1sb, [X2a], tps, tmpp)

            # ====================================================== phase b2
            if PH >= 5:
             with ExitStack() as c2:
                p_b2 = c2.enter_context(tc.tile_pool(name="p_b2", bufs=1))
                Wsb2 = p_b2.tile([128, DCOLS[1][1]], cdt, tag="Wsb2")
                nc.sync.dma_start(out=Wsb2[:], in_=Wd_d[1][:])
                T2sb = p_b2.tile([128, TCOLS[1][1]], cdt, tag="T2sb")
                nc.sync.dma_start(out=T2sb[:], in_=Tw_d[1][:])
                X2b = p_b2.tile([128, 66, 66], cdt, tag="X2b")
                X2c = p_b2.tile([3, 66, 66], cdt, tag="X2c")
                memset_borders(X2b, 64)
                memset_borders(X2c, 64)
                with tc.tile_pool(name="dps2", bufs=2, space="PSUM") as dps, \
                        tc.tile_pool(name="stg2", bufs=3) as stg:
                    _dense_block(nc, tc, 1, [X2a, X2b, X2c], Wsb2, dps, stg)
                with tc.tile_pool(name="tps2", bufs=2, space="PSUM") as tps, \
                        tc.tile_pool(name="tmp2", bufs=2) as tmpp:
                    _transition(nc, tc, 1, [X2a, X2b, X2c], T2sb, [X3a, X3b], tps, tmpp)

            # ====================================================== phase b3
            if PH >= 6:
             with ExitStack() as c3:
                p_b3 = c3.enter_context(tc.tile_pool(name="p_b3", bufs=1))
                Wsb3 = p_b3.tile([128, DCOLS[2][1]], cdt, tag="Wsb3")
                nc.sync.dma_start(out=Wsb3[:], in_=Wd_d[2][:])
                T3sb = p_b3.tile([128, TCOLS[2][1]], cdt, tag="T3sb")
                nc.sync.dma_start(out=T3sb[:], in_=Tw_d[2][:])
                X3c = p_b3.tile([33, 34, 34], cdt, tag="X3c")
                memset_borders(X3c, 32)
                X4a = p_b3.tile([128, 16, 16], f32, tag="X4a")
                X4b = p_b3.tile([16, 16, 16], f32, tag="X4b")
                with tc.tile_pool(name="dps3", bufs=2, space="PSUM") as dps, \
                        tc.tile_pool(name="stg3", bufs=3) as stg:
                    _dense_block(nc, tc, 2, [X3a, X3b, X3c], Wsb3, dps, stg)
                if PH >= 7:
                    with tc.tile_pool(name="tps3", bufs=2, space="PSUM") as tps, \
                            tc.tile_pool(name="tmp3", bufs=2) as tmpp:
                        _transition(nc, tc, 2, [X3a, X3b, X3c], T3sb, [X4a, X4b], tps, tmpp)

                # --------------------------------------------------- FC head
                if PH >= 8:
                 with nc.named_scope("fc"):
                    Wfc_sb = p_b3.tile([128, 2, 2, 256], f32, tag="Wfc")
                    nc.sync.dma_start(out=Wfc_sb[:], in_=Wfc_d[:])
                    bfc_sb = p_b3.tile([1, 2], f32, tag="bfc")
                    nc.sync.dma_start(out=bfc_sb[:], in_=bfc_d[:])
                    P4 = p_b3.tile([128, 4], f32, tag="P4")
                    nc.vector.memset(P4[:], 0.0)
                    scr = p_b3.tile([128, 256], f32, tag="fscr")
                    for j in range(2):
                        for t, rows in ((0, 128), (1, 16)):
                            X4t = (X4a, X4b)[t]
                            nc.vector.tensor_mul(
                                scr[0:rows, :],
                                X4t[0:rows].rearrange("c r x -> c (r x)"),
                                Wfc_sb[0:rows, j, t, :],
                            )
                            nc.vector.reduce_sum(
                                P4[0:rows, 2 * j + t : 2 * j + t + 1],
                                scr[0:rows, :],
                                axis=mybir.AxisListType.X,
                            )
                    ones128 = p_b3.tile([128, 1], f32, tag="ones128")
                    nc.vector.memset(ones128[:], 1.0)
                    fps = c3.enter_context(tc.tile_pool(name="fps", bufs=1, space="PSUM"))
                    psf = fps.tile([1, 4], f32, tag="psf")
                    nc.tensor.matmul(
                        psf[0:1, :], ones128[:, 0:1], P4[:], start=True, stop=True
                    )
                    f4 = p_b3.tile([1, 4], f32, tag="f4")
                    nc.vector.tensor_copy(f4[:], psf[0:1, :])
                    out2 = p_b3.tile([1, 2], f32, tag="out2")
                    nc.vector.tensor_add(out2[:], f4[0:1, 0:4:2], f4[0:1, 1:4:2])
                    nc.vector.tensor_add(out2[:], out2[:], bfc_sb[:])
                    nc.sync.dma_start(out=out_d[:], in_=out2[:])

    if PH < 8:
        with tile.TileContext(nc) as tc2:
            with tc2.tile_pool(name="fallout", bufs=1) as fo:
                z = fo.tile([1, 2], f32, tag="zout")
                nc.vector.memset(z[:], 0.0)
                nc.sync.dma_start(out=out_d[:], in_=z[:])
    nc.finalize()
    return nc


def _dense_block(nc, tc, bl, Xt, Wsb, dps, stg):
    import concourse.mybir as mybir

    f32 = mybir.dt.float32
    cdt = mybir.dt.bfloat16 if CONV_DT == "bf16" else mybir.dt.float32
    AF = mybir.ActivationFunctionType
    spec = BLK[bl]
    W, caps, rpc = spec["W"], spec["caps"], spec["rpc"]
    N = rpc * W
    cols = DCOLS[bl][0]
    for l in range(NL):
        cin = spec["cin0"] + GR * l
        chunks = _chunks_of(cin, caps)
        outs = _pieces_of(cin, cin + GR, caps)
        n_mm = 9 * len(chunks)
        with nc.named_scope(f"b{bl + 1}_l{l}"):
            for g in range(spec["groups"]):
                psl = [
                    dps.tile([128, N], f32, tag=f"dps{cg}", name=f"dps{bl}_{cg}")
                    for cg in range(4)
                ]
                i = 0
                for ci, (ti, kc) in enumerate(chunks):
                    for ky in (-1, 0, 1):
                        for kx in (-1, 0, 1):
                            col = cols[(l, ci, ky, kx)]
                            for cg in range(4):
                                r0 = (g * 4 + cg) * rpc
                                rhs = Xt[ti][
                                    0:kc,
                                    r0 + 1 + ky : r0 + 1 + ky + rpc,
                                    1 + kx : 1 + kx + W,
                                ]
                                nc.tensor.matmul(
                                    psl[cg][32 * cg : 32 * cg + GR, :],
                                    Wsb[0:kc, col : col + GR],
                                    rhs,
                                    start=(i == 0),
                                    stop=(i == n_mm - 1),
                                    tile_position=(0, 32 * cg),
                                )
                            i += 1
                st = stg.tile([32, 4, N], cdt, tag="st")
                for cg in range(4):
                    src = psl[cg][32 * cg : 32 * cg + GR, :]
                    dst = st[0:GR, cg, :]
                    if cg % 2 == 0:
                        nc.scalar.activation(dst, src, AF.Relu)
                    else:
                        nc.vector.tensor_relu(dst, src)
                r0 = g * 4 * rpc
                for ti, lo, off, n in outs:
                    nc.sync.dma_start(
                        out=Xt[ti][lo : lo + n, r0 + 1 : r0 + 1 + 4 * rpc, 1 : 1 + W],
                        in_=st[off : off + n].rearrange(
                            "c g (r x) -> c (g r) x", x=W
                        ),
                    )


def _transition(nc, tc, bl, Xt, Tsb, Xn, tps, tmpp):
    import concourse.mybir as mybir

    f32 = mybir.dt.float32
    spec = TRS[bl]
    W = spec["W"]
    rpg = 512 // W
    ngr = W // rpg
    chunks = _chunks_of(spec["cin"], BLK[bl]["caps"])
    cols = TCOLS[bl][0]
    padded_out = bl < 2  # t3 output (X4) is unpadded
    with nc.named_scope(f"trans{bl + 1}"):
        for g in range(ngr):
            r0 = g * rpg
            psl = []
            for mi, (mlo, mn) in enumerate(spec["mts"]):
                psl.append(
                    tps.tile([128, 512], f32, tag=f"tps{mi}", name=f"tps{bl}_{mi}")
                )
            for ci, (ti, kc) in enumerate(chunks):
                rhs = Xt[ti][0:kc, r0 + 1 : r0 + 1 + rpg, 1 : 1 + W]
                for mi, (mlo, mn) in enumerate(spec["mts"]):
                    col = cols[(ci, mi)]
                    nc.tensor.matmul(
                        psl[mi][0:mn, :],
                        Tsb[0:kc, col : col + mn],
                        rhs,
                        start=(ci == 0),
                        stop=(ci == len(chunks) - 1),
                    )
            for mi, (mlo, mn) in enumerate(spec["mts"]):
                psv = psl[mi][0:mn, :].rearrange(
                    "c (r x two) -> c r x two", r=rpg, two=2
                )
                tmp = tmpp.tile([128, rpg, W // 2], f32, tag="pooltmp")
                nc.vector.tensor_reduce(
                    tmp[0:mn],
                    psv,
                    axis=mybir.AxisListType.X,
                    op=mybir.AluOpType.max,
                )
                ro = r0 // 2
                dst_t = Xn[mi]
                if padded_out:
                    dst = dst_t[0:mn, ro + 1 : ro + 1 + rpg // 2, 1 : 1 + W // 2]
                else:
                    dst = dst_t[0:mn, ro : ro + rpg // 2, :]
                nc.vector.tensor_max(
                    dst, tmp[0:mn, 0 : rpg : 2, :], tmp[0:mn, 1 : rpg : 2, :]
                )


# ----------------------------------------------------------------- host side
def _pack_weights(inputs):
    import ml_dtypes
    f = np.float32
    cnp = ml_dtypes.bfloat16 if CONV_DT == "bf16" else np.float32
    Wt = inputs["Wt"]  # (D, E)
    WtT = np.ascontiguousarray(Wt.T, dtype=f)  # (E, D)
    bt = inputs["bt"].reshape(D, 1).astype(f)
    w_attn = inputs["w_attn"]
    wup = w_attn[0:D].reshape(D, 1).astype(f)
    wprod = w_attn[2 * D : 3 * D].reshape(D, 1).astype(f)
    WcT = np.ascontiguousarray(inputs["Wc_in"][:, :, 0, 0].T, dtype=f)  # (D, C0)
    biascol = np.zeros((128, 1), f)
    biascol[0:C0, 0] = inputs["bc_in"]
    biascol[64 : 64 + C0, 0] = inputs["bc_in"]

    Wd = []
    for bl in range(3):
        cols, total = DCOLS[bl]
        ws = inputs[f"dense{bl + 1}_ws"]
        arr = np.zeros((128, total), f)
        spec = BLK[bl]
        for l in range(NL):
            w = np.asarray(ws[l])  # (GR, cin, 3, 3)
            cin = spec["cin0"] + GR * l
            for ci, (ti, kc) in enumerate(_chunks_of(cin, spec["caps"])):
                base = 128 * ti
                for iky, ky in enumerate((-1, 0, 1)):
                    for ikx, kx in enumerate((-1, 0, 1)):
                        c = cols[(l, ci, ky, kx)]
                        # lhsT[r, m] = w[m, base + r, iky, ikx]
                        arr[0:kc, c : c + GR] = w[:, base : base + kc, iky, ikx].T
        Wd.append(arr)

    Tw = []
    for bl in range(3):
        cols, total = TCOLS[bl]
        tw = np.asarray(inputs[f"trans{bl + 1}_w"])  # (nout, cin, 1, 1)
        arr = np.zeros((128, total), f)
        for ci, (ti, kc) in enumerate(_chunks_of(TRS[bl]["cin"], BLK[bl]["caps"])):
            base = 128 * ti
            for mi, (mlo, mn) in enumerate(TRS[bl]["mts"]):
                c = cols[(ci, mi)]
                arr[0:kc, c : c + mn] = tw[mlo : mlo + mn, base : base + kc, 0, 0].T
        Tw.append(arr)

    Wfc = np.asarray(inputs["Wfc"])  # (2, 36864)
    Wfc_arr = np.zeros((128, 2, 2, 256), f)
    wv = Wfc.reshape(2, 144, 256)
    for t, rows in ((0, 128), (1, 16)):
        Wfc_arr[0:rows, :, t, :] = np.transpose(wv[:, 128 * t : 128 * t + rows, :], (1, 0, 2))
    bfc = inputs["bfc"].reshape(1, 2).astype(f)

    return dict(
        WtT=WtT, bt=bt, wup=wup, wprod=wprod, WcT=WcT.astype(cnp),
        biascol=biascol,
        zeros=np.zeros((128, 132), cnp),
        Wd0=Wd[0].astype(cnp), Wd1=Wd[1].astype(cnp), Wd2=Wd[2].astype(cnp),
        Tw0=Tw[0].astype(cnp), Tw1=Tw[1].astype(cnp), Tw2=Tw[2].astype(cnp),
        Wfc=Wfc_arr, bfc=bfc,
    )


_NC_CACHE = {}


def kernel(**inputs):
    from concourse.bass_utils import run_bass_kernel_spmd

    if "nc" not in _NC_CACHE:
        _NC_CACHE["nc"] = _build_nc()
    nc = _NC_CACHE["nc"]

    shared = _pack_weights(inputs)
    p_emb = np.asarray(inputs["p_emb"], dtype=np.float32)
    h_emb = np.asarray(inputs["h_emb"], dtype=np.float32)
    in_maps = []
    for c in range(B):
        m = dict(shared)
        m["pT"] = np.ascontiguousarray(p_emb[c].T)
        m["hT"] = np.ascontiguousarray(h_emb[c].T)
        in_maps.append(m)

    res = run_bass_kernel_spmd(nc, in_maps, core_ids=list(range(B)))
    out = np.stack([r["out"].reshape(2) for r in res.results], axis=0)
    return out.astype(np.float32)


if __name__ == "__main__":
    nc = _build_nc()
    print("built ok")


# revision 26
# speedup vs baseline: 1.0083x; 1.0083x over previous
"""DIIN Trainium2 Bass kernel.

Data-parallel over batch (B=8 -> 8 cores, one batch element per core).
Per core: self-attention encode of p/h (fp32), interaction tensor,
1x1 conv-in, 3 DenseNet blocks (8 conv3x3 layers each, growth 20) with
1x1-conv + 2x2-maxpool transitions, final FC -> 2 logits (fp32).

Conv3x3 mapping: weight-stationary matmuls with the 9 kernel offsets as
separate PSUM-accumulated matmuls; the shifted input windows are plain
strided access patterns over zero-padded [C, H+2, W+2] SBUF feature maps
(no im2col materialization). Output channels (20) use only a sliver of
the PE array column dim, so each spatial group is split into four chunks
packed onto the four 32-column groups of the PE array (tile_position via
the psum base partition); the four streams execute concurrently on
disjoint 32x32 sub-array columns. Each column group accumulates in its
own PSUM bank (a single accumulation group must not span column groups:
start=True clears has_written for the whole bank).

The conv path runs in bf16 (fp32 PSUM accumulation): fp32 matmuls stream
at 4 cycles/row on TRN2 vs 1 for bf16, and fp32r (1 cycle/row at N>=256)
requires dst partition 0, which defeats the column packing. Set
CONV_DT=f32 for a bit-accurate (~4e-7 rel err) but ~3-4x slower variant;
bf16 measures ~2e-3 rel err end-to-end.

Epilogue: per group, 4 relu ops (ScalarE/VectorE alternating) compact the
per-column-group PSUM slices to partition base 0 of a [32, 4, N] staging
tile (engine SBUF accesses must start at 32-aligned partitions), then one
SBUF->SBUF DMA scatters the 20 new channels into the padded feature map
at their (arbitrary, DMA-only) partition rows - the in-place DenseNet
concat.
"""

import os
import sys
import numpy as np

if "/opt/trn_rl_repo" not in sys.path:
    sys.path.insert(0, "/opt/trn_rl_repo")

CONV_DT = os.environ.get("CONV_DT", "bf16")  # "bf16" or "f32"

# ----------------------------------------------------------------- constants
B, P, E, D = 8, 128, 768, 128
GR, NL = 20, 8
C0 = 38

# per block: spatial W(=H), X-tile channel capacities, input channels of layer0,
# psum spatial groups, rows per col-group chunk
BLK = [
    dict(W=128, caps=[128, 70], cin0=38, groups=8, rpc=4),
    dict(W=64, caps=[128, 128, 3], cin0=99, groups=2, rpc=8),
    dict(W=32, caps=[128, 128, 33], cin0=129, groups=1, rpc=8),
]
# transitions: input channels, output M-tiles [(chan_lo, n)], input spatial W
TRS = [
    dict(cin=198, mts=[(0, 99)], W=128),
    dict(cin=259, mts=[(0, 128), (128, 1)], W=64),
    dict(cin=289, mts=[(0, 128), (128, 16)], W=32),
]


def _chunks_of(cin, caps):
    """contraction chunks [(tile_idx, kc)] covering channels [0, cin)"""
    out = []
    for ti, cap in enumerate(caps):
        base = 128 * ti
        if base >= cin:
            break
        out.append((ti, min(cin - base, cap)))
    return out


def _pieces_of(lo, hi, caps):
    """[(tile_idx, local_lo, off_in_range, n)] covering channels [lo, hi)"""
    out = []
    for ti, cap in enumerate(caps):
        base = 128 * ti
        a, b = max(lo, base), min(hi, base + cap)
        if a < b:
            out.append((ti, a - base, a - lo, b - a))
    return out


def _dense_cols(bl):
    """column offsets for the packed dense-conv weight tensor of block bl"""
    cols = {}
    c = 0
    spec = BLK[bl]
    for l in range(NL):
        cin = spec["cin0"] + GR * l
        for ci, (ti, kc) in enumerate(_chunks_of(cin, spec["caps"])):
            for ky in (-1, 0, 1):
                for kx in (-1, 0, 1):
                    cols[(l, ci, ky, kx)] = c
                    c += GR
    return cols, c


def _trans_cols(bl):
    cols = {}
    c = 0
    spec = TRS[bl]
    for ci, (ti, kc) in enumerate(_chunks_of(spec["cin"], BLK[bl]["caps"])):
        for mi, (mlo, mn) in enumerate(spec["mts"]):
            cols[(ci, mi)] = c
            c += mn
    return cols, c


DCOLS = [_dense_cols(b) for b in range(3)]
TCOLS = [_trans_cols(b) for b in range(3)]


# ----------------------------------------------------------------- bass build
def _build_nc():
    import os
    PH = int(os.environ.get("KERNEL_PHASES", "9"))
    import concourse.bacc as bacc
    import concourse.tile as tile
    import concourse.mybir as mybir
    from concourse.masks import make_identity

    f32 = mybir.dt.float32
    cdt = mybir.dt.bfloat16 if CONV_DT == "bf16" else mybir.dt.float32
    AF = mybir.ActivationFunctionType
    ALU = mybir.AluOpType
    AX = mybir.AxisListType

    nc = bacc.Bacc(None, target_bir_lowering=False)

    # ---- dram I/O
    pT_d = nc.dram_tensor("pT", [E, P], f32, kind="ExternalInput")
    hT_d = nc.dram_tensor("hT", [E, P], f32, kind="ExternalInput")
    WtT_d = nc.dram_tensor("WtT", [E, D], f32, kind="ExternalInput")
    bt_d = nc.dram_tensor("bt", [D, 1], f32, kind="ExternalInput")
    wup_d = nc.dram_tensor("wup", [D, 1], f32, kind="ExternalInput")
    wprod_d = nc.dram_tensor("wprod", [D, 1], f32, kind="ExternalInput")
    WcT_d = nc.dram_tensor("WcT", [D, C0], cdt, kind="ExternalInput")
    biascol_d = nc.dram_tensor("biascol", [128, 1], f32, kind="ExternalInput")
    Wd_d = [
        nc.dram_tensor(f"Wd{i}", [128, DCOLS[i][1]], cdt, kind="ExternalInput")
        for i in range(3)
    ]
    Tw_d = [
        nc.dram_tensor(f"Tw{i}", [128, TCOLS[i][1]], cdt, kind="ExternalInput")
        for i in range(3)
    ]
    Wfc_d = nc.dram_tensor("Wfc", [128, 2, 2, 256], f32, kind="ExternalInput")
    bfc_d = nc.dram_tensor("bfc", [1, 2], f32, kind="ExternalInput")
    zeros_d = nc.dram_tensor("zeros", [128, 132], cdt, kind="ExternalInput")
    out_d = nc.dram_tensor("out", [1, 2], f32, kind="ExternalOutput")

    with tile.TileContext(nc) as tc:
        from contextlib import ExitStack

        with ExitStack() as ctx:
            persist = ctx.enter_context(tc.tile_pool(name="persist", bufs=1))
            # persistent across block boundaries
            X2a = persist.tile([128, 66, 66], cdt, tag="X2a")
            X3a = persist.tile([128, 34, 34], cdt, tag="X3a")
            X3b = persist.tile([128, 34, 34], cdt, tag="X3b")

            def memset_borders(t, w):
                n = t.shape[0]
                nc.sync.dma_start(out=t[:, 0, :], in_=zeros_d[0:n, 0 : w + 2])
                nc.sync.dma_start(out=t[:, w + 1, :], in_=zeros_d[0:n, 0 : w + 2])
                nc.sync.dma_start(out=t[:, :, 0], in_=zeros_d[0:n, 0 : w + 2])
                nc.sync.dma_start(out=t[:, :, w + 1], in_=zeros_d[0:n, 0 : w + 2])

            # ====================================================== phase b1
            with ExitStack() as c1:
                p_b1 = c1.enter_context(tc.tile_pool(name="p_b1", bufs=1))
                Wsb1 = p_b1.tile([128, DCOLS[0][1]], cdt, tag="Wsb1")
                nc.sync.dma_start(out=Wsb1[:], in_=Wd_d[0][:])
                T1sb = p_b1.tile([128, TCOLS[0][1]], cdt, tag="T1sb")
                nc.sync.dma_start(out=T1sb[:], in_=Tw_d[0][:])
                WcT_sb = p_b1.tile([D, C0], cdt, tag="WcT")
                nc.sync.dma_start(out=WcT_sb[:], in_=WcT_d[:])
                biascol = p_b1.tile([128, 1], f32, tag="biascol")
                nc.sync.dma_start(out=biascol[:], in_=biascol_d[:])
                X1a = p_b1.tile([128, 130, 130], cdt, tag="X1a")
                peT = p_b1.tile([D, P], f32, tag="peT")
                heT = p_b1.tile([D, P], f32, tag="heT")
                memset_borders(X1a, 128)
                memset_borders(X2a, 64)
                memset_borders(X3a, 32)
                memset_borders(X3b, 32)

                # ---------------------------------------------- encoders
                with ExitStack() as ce:
                    enc = ce.enter_context(tc.tile_pool(name="enc", bufs=2))
                    encps = ce.enter_context(
                        tc.tile_pool(name="encps", bufs=2, space="PSUM")
                    )
                    consts = ce.enter_context(tc.tile_pool(name="consts", bufs=1))
                    ident = consts.tile([128, 128], f32, tag="ident")
                    make_identity(nc, ident[:])
                    WtT_sb = consts.tile([128, 6, 128], f32, tag="WtT")
                    nc.sync.dma_start(
                        out=WtT_sb[:], in_=WtT_d.rearrange("(k p) j -> p k j", p=128)
                    )
                    bt_sb = consts.tile([D, 1], f32, tag="bt")
                    nc.sync.dma_start(out=bt_sb[:], in_=bt_d[:])
                    wup_sb = consts.tile([D, 1], f32, tag="wup")
                    nc.sync.dma_start(out=wup_sb[:], in_=wup_d[:])
                    wprod_sb = consts.tile([D, 1], f32, tag="wprod")
                    nc.sync.dma_start(out=wprod_sb[:], in_=wprod_d[:])
                    ones1 = consts.tile([1, 128], f32, tag="ones1")
                    nc.vector.memset(ones1[:], 1.0)

                    for name, src_d, dst in (("p", pT_d, peT), ("h", hT_d, heT)):
                        with nc.named_scope(f"enc_{name}"):
                            XE = enc.tile([128, 6, 128], f32, tag="XE")
                            nc.sync.dma_start(
                                out=XE[:], in_=src_d.rearrange("(k p) j -> p k j", p=128)
                            )
                            psxT = encps.tile([128, 128], f32, tag="psxT")
                            for k in range(6):
                                nc.tensor.matmul(
                                    psxT[:],
                                    WtT_sb[:, k, :],
                                    XE[:, k, :],
                                    start=(k == 0),
                                    stop=(k == 5),
                                )
                            xT = enc.tile([128, 128], f32, tag="xT")
                            nc.scalar.activation(
                                xT[:], psxT[:], AF.Identity, bias=bt_sb[:]
                            )
                            # x (token-major) via PE transpose
                            psT = encps.tile([128, 128], f32, tag="psT")
                            nc.tensor.transpose(psT[:], xT[:], ident[:])
                            x_sb = enc.tile([128, 128], f32, tag="x_sb")
                            nc.vector.tensor_copy(x_sb[:], psT[:])
                            # u row
                            psu = encps.tile([1, 128], f32, tag="psu")
                            nc.tensor.matmul(
                                psu[0:1, :], wup_sb[:, 0:1], xT[:], start=True, stop=True
                            )
                            u_sb = enc.tile([1, 128], f32, tag="u_sb")
                            nc.vector.tensor_copy(u_sb[:], psu[0:1, :])
                            # A = (x*wprod) @ x.T + u[None, :]
                            xwT = enc.tile([128, 128], f32, tag="xwT")
                            nc.vector.tensor_scalar_mul(xwT[:], xT[:], wprod_sb[:, 0:1])
                            psA = encps.tile([128, 128], f32, tag="psA")
                            nc.tensor.matmul(psA[:], xwT[:], xT[:], start=True, stop=False)
                            nc.tensor.matmul(
                                psA[:], ones1[0:1, :], u_sb[0:1, :], start=False, stop=True
                            )
                            # softmax over free dim
                            negmax = enc.tile([128, 1], f32, tag="negmax")
                            nc.vector.tensor_reduce(
                                negmax[:], psA[:], axis=AX.X, op=ALU.max, negate=True
                            )
                            sAe = enc.tile([128, 128], f32, tag="sAe")
                            den = enc.tile([128, 1], f32, tag="den")
                            nc.scalar.activation(
                                sAe[:], psA[:], AF.Exp, bias=negmax[:], scale=1.0,
                                accum_out=den[:],
                            )
                            rden = enc.tile([128, 1], f32, tag="rden")
                            nc.vector.reciprocal(rden[:], den[:])
                            sAn = enc.tile([128, 128], f32, tag="sAn")
                            nc.vector.tensor_scalar_mul(sAn[:], sAe[:], rden[:])
                            psT2 = encps.tile([128, 128], f32, tag="psT")
                            nc.tensor.transpose(psT2[:], sAn[:], ident[:])
                            sAT = enc.tile([128, 128], f32, tag="sAT")
                            nc.vector.tensor_copy(sAT[:], psT2[:])
                            pspeT = encps.tile([128, 128], f32, tag="psA")
                            nc.tensor.matmul(
                                pspeT[:], x_sb[:], sAT[:], start=True, stop=True
                            )
                            nc.scalar.copy(dst[:], pspeT[:])

                # ------------------------------------- interaction + conv_in
                if PH >= 2:
                 with ExitStack() as ci:
                    pI = ci.enter_context(tc.tile_pool(name="pI", bufs=1))
                    cips = ci.enter_context(
                        tc.tile_pool(name="cips", bufs=2, space="PSUM")
                    )
                    stg0 = ci.enter_context(tc.tile_pool(name="stg0", bufs=3))
                    I = pI.tile([128, 128, 128], cdt, tag="I")  # [d, p, h]
                    with nc.named_scope("interaction"):
                        for p in range(P):
                            nc.vector.tensor_scalar_mul(
                                I[:, p, :], heT[:], peT[:, p : p + 1]
                            )
                    with nc.named_scope("conv_in"):
                        for g in range(16):
                            psl = [
                                cips.tile([128, 512], f32, tag=f"cip{cg}",
                                          name=f"cip{cg}")
                                for cg in range(2)
                            ]
                            for cg in range(2):
                                rhs = I[:, g * 8 + cg * 4 : g * 8 + cg * 4 + 4, :]
                                nc.tensor.matmul(
                                    psl[cg][64 * cg : 64 * cg + C0, :],
                                    WcT_sb[:],
                                    rhs,
                                    start=True,
                                    stop=True,
                                    tile_position=(0, 64 * cg),
                                )
                            st0 = stg0.tile([64, 2, 512], cdt, tag="st0")
                            for cg in range(2):
                                src = psl[cg][64 * cg : 64 * cg + C0, :]
                                dst = st0[0:C0, cg, :]
                                nc.scalar.activation(
                                    dst, src, AF.Identity, bias=biascol[0:C0, :]
                                )
                            nc.sync.dma_start(
                                out=X1a[0:C0, g * 8 + 1 : g * 8 + 9, 1:129],
                                in_=st0[0:C0].rearrange(
                                    "c g (r x) -> c (g r) x", x=128
                                ),
                            )

                # ------------------------------------------- dense block 1
                if PH >= 3:
                 with ExitStack() as cb:
                    px1b = cb.enter_context(tc.tile_pool(name="px1b", bufs=1))
                    X1b = px1b.tile([70, 130, 130], cdt, tag="X1b")
                    memset_borders(X1b, 128)
                    with tc.tile_pool(name="dps1", bufs=2, space="PSUM") as dps, \
                            tc.tile_pool(name="stg1", bufs=3) as stg:
                        _dense_block(nc, tc, 0, [X1a, X1b], Wsb1, dps, stg)
                    # transition 1 -> X2a
                    if PH >= 4:
                     with tc.tile_pool(name="tps1", bufs=2, space="PSUM") as tps, \
                            tc.tile_pool(name="tmp1", bufs=2) as tmpp:
                        _transition(nc, tc, 0, [X1a, X1b], T1sb, [X2a], tps, tmpp)

            # ====================================================== phase b2
            if PH >= 5:
             with ExitStack() as c2:
                p_b2 = c2.enter_context(tc.tile_pool(name="p_b2", bufs=1))
                Wsb2 = p_b2.tile([128, DCOLS[1][1]], cdt, tag="Wsb2")
                nc.sync.dma_start(out=Wsb2[:], in_=Wd_d[1][:])
                T2sb = p_b2.tile([128, TCOLS[1][1]], cdt, tag="T2sb")
                nc.sync.dma_start(out=T2sb[:], in_=Tw_d[1][:])
                X2b = p_b2.tile([128, 66, 66], cdt, tag="X2b")
                X2c = p_b2.tile([3, 66, 66], cdt, tag="X2c")
                memset_borders(X2b, 64)
                memset_borders(X2c, 64)
                with tc.tile_pool(name="dps2", bufs=2, space="PSUM") as dps, \
                        tc.tile_pool(name="stg2", bufs=3) as stg:
                    _dense_block(nc, tc, 1, [X2a, X2b, X2c], Wsb2, dps, stg)
                with tc.tile_pool(name="tps2", bufs=2, space="PSUM") as tps, \
                        tc.tile_pool(name="tmp2", bufs=2) as tmpp:
                    _transition(nc, tc, 1, [X2a, X2b, X2c], T2sb, [X3a, X3b], tps, tmpp)

            # ====================================================== phase b3
            if PH >= 6:
             with ExitStack() as c3:
                p_b3 = c3.enter_context(tc.tile_pool(name="p_b3", bufs=1))
                Wsb3 = p_b3.tile([128, DCOLS[2][1]], cdt, tag="Wsb3")
                nc.sync.dma_start(out=Wsb3[:], in_=Wd_d[2][:])
                T3sb = p_b3.tile([128, TCOLS[2][1]], cdt, tag="T3sb")
                nc.sync.dma_start(out=T3sb[:], in_=Tw_d[2][:])
                X3c = p_b3.tile([33, 34, 34], cdt, tag="X3c")
                memset_borders(X3c, 32)
                X4a = p_b3.tile([128, 16, 16], f32, tag="X4a")
                X4b = p_b3.tile([16, 16, 16], f32, tag="X4b")
                with tc.tile_pool(name="dps3", bufs=2, space="PSUM") as dps, \
                        tc.tile_pool(name="stg3", bufs=3) as stg:
                    _dense_block(nc, tc, 2, [X3a, X3b, X3c], Wsb3, dps, stg)
                if PH >= 7:
                    with tc.tile_pool(name="tps3", bufs=2, space="PSUM") as tps, \
                            tc.tile_pool(name="tmp3", bufs=2) as tmpp:
                        _transition(nc, tc, 2, [X3a, X3b, X3c], T3sb, [X4a, X4b], tps, tmpp)

                # --------------------------------------------------- FC head
                if PH >= 8:
                 with nc.named_scope("fc"):
                    Wfc_sb = p_b3.tile([128, 2, 2, 256], f32, tag="Wfc")
                    nc.sync.dma_start(out=Wfc_sb[:], in_=Wfc_d[:])
                    bfc_sb = p_b3.tile([1, 2], f32, tag="bfc")
                    nc.sync.dma_start(out=bfc_sb[:], in_=bfc_d[:])
                    P4 = p_b3.tile([128, 4], f32, tag="P4")
                    nc.vector.memset(P4[:], 0.0)
                    scr = p_b3.tile([128, 256], f32, tag="fscr")
                    for j in range(2):
                        for t, rows in ((0, 128), (1, 16)):
                            X4t = (X4a, X4b)[t]
                            nc.vector.tensor_mul(
                                scr[0:rows, :],
                                X4t[0:rows].rearrange("c r x -> c (r x)"),
                                Wfc_sb[0:rows, j, t, :],
                            )
                            nc.vector.reduce_sum(
                                P4[0:rows, 2 * j + t : 2 * j + t + 1],
                                scr[0:rows, :],
                                axis=mybir.AxisListType.X,
                            )
                    ones128 = p_b3.tile([128, 1], f32, tag="ones128")
                    nc.vector.memset(ones128[:], 1.0)
                    fps = c3.enter_context(tc.tile_pool(name="fps", bufs=1, space="PSUM"))
                    psf = fps.tile([1, 4], f32, tag="psf")
                    nc.tensor.matmul(
                        psf[0:1, :], ones128[:, 0:1], P4[:], start=True, stop=True
                    )
                    f4 = p_b3.tile([1, 4], f32, tag="f4")
                    nc.vector.tensor_copy(f4[:], psf[0:1, :])
                    out2 = p_b3.tile([1, 2], f32, tag="out2")
                    nc.vector.tensor_add(out2[:], f4[0:1, 0:4:2], f4[0:1, 1:4:2])
                    nc.vector.tensor_add(out2[:], out2[:], bfc_sb[:])
                    nc.sync.dma_start(out=out_d[:], in_=out2[:])

    if PH < 8:
        with tile.TileContext(nc) as tc2:
            with tc2.tile_pool(name="fallout", bufs=1) as fo:
                z = fo.tile([1, 2], f32, tag="zout")
                nc.vector.memset(z[:], 0.0)
                nc.sync.dma_start(out=out_d[:], in_=z[:])
    nc.finalize()
    return nc


def _dense_block(nc, tc, bl, Xt, Wsb, dps, stg):
    import concourse.mybir as mybir

    f32 = mybir.dt.float32
    cdt = mybir.dt.bfloat16 if CONV_DT == "bf16" else mybir.dt.float32
    AF = mybir.ActivationFunctionType
    spec = BLK[bl]
    W, caps, rpc = spec["W"], spec["caps"], spec["rpc"]
    N = rpc * W
    cols = DCOLS[bl][0]
    for l in range(NL):
        cin = spec["cin0"] + GR * l
        chunks = _chunks_of(cin, caps)
        outs = _pieces_of(cin, cin + GR, caps)
        n_mm = 9 * len(chunks)
        with nc.named_scope(f"b{bl + 1}_l{l}"):
            for g in range(spec["groups"]):
                psl = [
                    dps.tile([128, N], f32, tag=f"dps{cg}", name=f"dps{bl}_{cg}")
                    for cg in range(4)
                ]
                i = 0
                for ci, (ti, kc) in enumerate(chunks):
                    for ky in (-1, 0, 1):
                        for kx in (-1, 0, 1):
                            col = cols[(l, ci, ky, kx)]
                            for cg in range(4):
                                r0 = (g * 4 + cg) * rpc
                                rhs = Xt[ti][
                                    0:kc,
                                    r0 + 1 + ky : r0 + 1 + ky + rpc,
                                    1 + kx : 1 + kx + W,
                                ]
                                nc.tensor.matmul(
                                    psl[cg][32 * cg : 32 * cg + GR, :],
                                    Wsb[0:kc, col : col + GR],
                                    rhs,
                                    start=(i == 0),
                                    stop=(i == n_mm - 1),
                                    tile_position=(0, 32 * cg),
                                )
                            i += 1
                st = stg.tile([32, 4, N], cdt, tag="st")
                for cg in range(4):
                    src = psl[cg][32 * cg : 32 * cg + GR, :]
                    dst = st[0:GR, cg, :]
                    if cg != 1:
                        nc.scalar.activation(dst, src, AF.Relu)
                    else:
                        nc.vector.tensor_relu(dst, src)
                r0 = g * 4 * rpc
                dma_eng = nc.sync if g % 2 == 0 else nc.scalar
                for ti, lo, off, n in outs:
                    dma_eng.dma_start(
                        out=Xt[ti][lo : lo + n, r0 + 1 : r0 + 1 + 4 * rpc, 1 : 1 + W],
                        in_=st[off : off + n].rearrange(
                            "c g (r x) -> c (g r) x", x=W
                        ),
                    )


def _transition(nc, tc, bl, Xt, Tsb, Xn, tps, tmpp):
    import concourse.mybir as mybir

    f32 = mybir.dt.float32
    spec = TRS[bl]
    W = spec["W"]
    rpg = 512 // W
    ngr = W // rpg
    chunks = _chunks_of(spec["cin"], BLK[bl]["caps"])
    cols = TCOLS[bl][0]
    padded_out = bl < 2  # t3 output (X4) is unpadded
    with nc.named_scope(f"trans{bl + 1}"):
        for g in range(ngr):
            r0 = g * rpg
            psl = []
            for mi, (mlo, mn) in enumerate(spec["mts"]):
                psl.append(
                    tps.tile([128, 512], f32, tag=f"tps{mi}", name=f"tps{bl}_{mi}")
                )
            for ci, (ti, kc) in enumerate(chunks):
                rhs = Xt[ti][0:kc, r0 + 1 : r0 + 1 + rpg, 1 : 1 + W]
                for mi, (mlo, mn) in enumerate(spec["mts"]):
                    col = cols[(ci, mi)]
                    nc.tensor.matmul(
                        psl[mi][0:mn, :],
                        Tsb[0:kc, col : col + mn],
                        rhs,
                        start=(ci == 0),
                        stop=(ci == len(chunks) - 1),
                    )
            for mi, (mlo, mn) in enumerate(spec["mts"]):
                psv = psl[mi][0:mn, :].rearrange(
                    "c (r x two) -> c r x two", r=rpg, two=2
                )
                tmp = tmpp.tile([128, rpg, W // 2], f32, tag="pooltmp")
                nc.vector.tensor_reduce(
                    tmp[0:mn],
                    psv,
                    axis=mybir.AxisListType.X,
                    op=mybir.AluOpType.max,
                )
                ro = r0 // 2
                dst_t = Xn[mi]
                if padded_out:
                    dst = dst_t[0:mn, ro + 1 : ro + 1 + rpg // 2, 1 : 1 + W // 2]
                else:
                    dst = dst_t[0:mn, ro : ro + rpg // 2, :]
                nc.vector.tensor_max(
                    dst, tmp[0:mn, 0 : rpg : 2, :], tmp[0:mn, 1 : rpg : 2, :]
                )


# ----------------------------------------------------------------- host side
def _pack_weights(inputs):
    import ml_dtypes
    f = np.float32
    cnp = ml_dtypes.bfloat16 if CONV_DT == "bf16" else np.float32
    Wt = inputs["Wt"]  # (D, E)
    WtT = np.ascontiguousarray(Wt.T, dtype=f)  # (E, D)
    bt = inputs["bt"].reshape(D, 1).astype(f)
    w_attn = inputs["w_attn"]
    wup = w_attn[0:D].reshape(D, 1).astype(f)
    wprod = w_attn[2 * D : 3 * D].reshape(D, 1).astype(f)
    WcT = np.ascontiguousarray(inputs["Wc_in"][:, :, 0, 0].T, dtype=f)  # (D, C0)
    biascol = np.zeros((128, 1), f)
    biascol[0:C0, 0] = inputs["bc_in"]
    biascol[64 : 64 + C0, 0] = inputs["bc_in"]

    Wd = []
    for bl in range(3):
        cols, total = DCOLS[bl]
        ws = inputs[f"dense{bl + 1}_ws"]
        arr = np.zeros((128, total), f)
        spec = BLK[bl]
        for l in range(NL):
            w = np.asarray(ws[l])  # (GR, cin, 3, 3)
            cin = spec["cin0"] + GR * l
            for ci, (ti, kc) in enumerate(_chunks_of(cin, spec["caps"])):
                base = 128 * ti
                for iky, ky in enumerate((-1, 0, 1)):
                    for ikx, kx in enumerate((-1, 0, 1)):
                        c = cols[(l, ci, ky, kx)]
                        # lhsT[r, m] = w[m, base + r, iky, ikx]
                        arr[0:kc, c : c + GR] = w[:, base : base + kc, iky, ikx].T
        Wd.append(arr)

    Tw = []
    for bl in range(3):
        cols, total = TCOLS[bl]
        tw = np.asarray(inputs[f"trans{bl + 1}_w"])  # (nout, cin, 1, 1)
        arr = np.zeros((128, total), f)
        for ci, (ti, kc) in enumerate(_chunks_of(TRS[bl]["cin"], BLK[bl]["caps"])):
            base = 128 * ti
            for mi, (mlo, mn) in enumerate(TRS[bl]["mts"]):
                c = cols[(ci, mi)]
                arr[0:kc, c : c + mn] = tw[mlo : mlo + mn, base : base + kc, 0, 0].T
        Tw.append(arr)

    Wfc = np.asarray(inputs["Wfc"])  # (2, 36864)
    Wfc_arr = np.zeros((128, 2, 2, 256), f)
    wv = Wfc.reshape(2, 144, 256)
    for t, rows in ((0, 128), (1, 16)):
        Wfc_arr[0:rows, :, t, :] = np.transpose(wv[:, 128 * t : 128 * t + rows, :], (1, 0, 2))
    bfc = inputs["bfc"].reshape(1, 2).astype(f)

    return dict(
        WtT=WtT, bt=bt, wup=wup, wprod=wprod, WcT=WcT.astype(cnp),
        biascol=biascol,
        zeros=np.zeros((128, 132), cnp),
        Wd0=Wd[0].astype(cnp), Wd1=Wd[1].astype(cnp), Wd2=Wd[2].astype(cnp),
        Tw0=Tw[0].astype(cnp), Tw1=Tw[1].astype(cnp), Tw2=Tw[2].astype(cnp),
        Wfc=Wfc_arr, bfc=bfc,
    )


_NC_CACHE = {}


def kernel(**inputs):
    from concourse.bass_utils import run_bass_kernel_spmd

    if "nc" not in _NC_CACHE:
        _NC_CACHE["nc"] = _build_nc()
    nc = _NC_CACHE["nc"]

    shared = _pack_weights(inputs)
    p_emb = np.asarray(inputs["p_emb"], dtype=np.float32)
    h_emb = np.asarray(inputs["h_emb"], dtype=np.float32)
    in_maps = []
    for c in range(B):
        m = dict(shared)
        m["pT"] = np.ascontiguousarray(p_emb[c].T)
        m["hT"] = np.ascontiguousarray(h_emb[c].T)
        in_maps.append(m)

    res = run_bass_kernel_spmd(nc, in_maps, core_ids=list(range(B)))
    out = np.stack([r["out"].reshape(2) for r in res.results], axis=0)
    return out.astype(np.float32)


if __name__ == "__main__":
    nc = _build_nc()
    print("built ok")


# revision 27
# speedup vs baseline: 1.0101x; 1.0018x over previous
"""DIIN Trainium2 Bass kernel.

Data-parallel over batch (B=8 -> 8 cores, one batch element per core).
Per core: self-attention encode of p/h (fp32), interaction tensor,
1x1 conv-in, 3 DenseNet blocks (8 conv3x3 layers each, growth 20) with
1x1-conv + 2x2-maxpool transitions, final FC -> 2 logits (fp32).

Conv3x3 mapping: weight-stationary matmuls with the 9 kernel offsets as
separate PSUM-accumulated matmuls; the shifted input windows are plain
strided access patterns over zero-padded [C, H+2, W+2] SBUF feature maps
(no im2col materialization). Output channels (20) use only a sliver of
the PE array column dim, so each spatial group is split into four chunks
packed onto the four 32-column groups of the PE array (tile_position via
the psum base partition); the four streams execute concurrently on
disjoint 32x32 sub-array columns. Each column group accumulates in its
own PSUM bank (a single accumulation group must not span column groups:
start=True clears has_written for the whole bank).

The conv path runs in bf16 (fp32 PSUM accumulation): fp32 matmuls stream
at 4 cycles/row on TRN2 vs 1 for bf16, and fp32r (1 cycle/row at N>=256)
requires dst partition 0, which defeats the column packing. Set
CONV_DT=f32 for a bit-accurate (~4e-7 rel err) but ~3-4x slower variant;
bf16 measures ~2e-3 rel err end-to-end.

Epilogue: per group, 4 relu ops (ScalarE/VectorE alternating) compact the
per-column-group PSUM slices to partition base 0 of a [32, 4, N] staging
tile (engine SBUF accesses must start at 32-aligned partitions), then one
SBUF->SBUF DMA scatters the 20 new channels into the padded feature map
at their (arbitrary, DMA-only) partition rows - the in-place DenseNet
concat.
"""

import os
import sys
import numpy as np

if "/opt/trn_rl_repo" not in sys.path:
    sys.path.insert(0, "/opt/trn_rl_repo")

CONV_DT = os.environ.get("CONV_DT", "bf16")  # "bf16" or "f32"

# ----------------------------------------------------------------- constants
B, P, E, D = 8, 128, 768, 128
GR, NL = 20, 8
C0 = 38

# per block: spatial W(=H), X-tile channel capacities, input channels of layer0,
# psum spatial groups, rows per col-group chunk
BLK = [
    dict(W=128, caps=[128, 70], cin0=38, groups=8, rpc=4),
    dict(W=64, caps=[128, 128, 3], cin0=99, groups=2, rpc=8),
    dict(W=32, caps=[128, 128, 33], cin0=129, groups=1, rpc=8),
]
# transitions: input channels, output M-tiles [(chan_lo, n)], input spatial W
TRS = [
    dict(cin=198, mts=[(0, 99)], W=128),
    dict(cin=259, mts=[(0, 128), (128, 1)], W=64),
    dict(cin=289, mts=[(0, 128), (128, 16)], W=32),
]


def _chunks_of(cin, caps):
    """contraction chunks [(tile_idx, kc)] covering channels [0, cin)"""
    out = []
    for ti, cap in enumerate(caps):
        base = 128 * ti
        if base >= cin:
            break
        out.append((ti, min(cin - base, cap)))
    return out


def _pieces_of(lo, hi, caps):
    """[(tile_idx, local_lo, off_in_range, n)] covering channels [lo, hi)"""
    out = []
    for ti, cap in enumerate(caps):
        base = 128 * ti
        a, b = max(lo, base), min(hi, base + cap)
        if a < b:
            out.append((ti, a - base, a - lo, b - a))
    return out


def _dense_cols(bl):
    """column offsets for the packed dense-conv weight tensor of block bl"""
    cols = {}
    c = 0
    spec = BLK[bl]
    for l in range(NL):
        cin = spec["cin0"] + GR * l
        for ci, (ti, kc) in enumerate(_chunks_of(cin, spec["caps"])):
            for ky in (-1, 0, 1):
                for kx in (-1, 0, 1):
                    cols[(l, ci, ky, kx)] = c
                    c += GR
    return cols, c


def _trans_cols(bl):
    cols = {}
    c = 0
    spec = TRS[bl]
    for ci, (ti, kc) in enumerate(_chunks_of(spec["cin"], BLK[bl]["caps"])):
        for mi, (mlo, mn) in enumerate(spec["mts"]):
            cols[(ci, mi)] = c
            c += mn
    return cols, c


DCOLS = [_dense_cols(b) for b in range(3)]
TCOLS = [_trans_cols(b) for b in range(3)]


# ----------------------------------------------------------------- bass build
def _build_nc():
    import os
    PH = int(os.environ.get("KERNEL_PHASES", "9"))
    import concourse.bacc as bacc
    import concourse.tile as tile
    import concourse.mybir as mybir
    from concourse.masks import make_identity

    f32 = mybir.dt.float32
    cdt = mybir.dt.bfloat16 if CONV_DT == "bf16" else mybir.dt.float32
    AF = mybir.ActivationFunctionType
    ALU = mybir.AluOpType
    AX = mybir.AxisListType

    nc = bacc.Bacc(None, target_bir_lowering=False)

    # ---- dram I/O
    pT_d = nc.dram_tensor("pT", [E, P], f32, kind="ExternalInput")
    hT_d = nc.dram_tensor("hT", [E, P], f32, kind="ExternalInput")
    WtT_d = nc.dram_tensor("WtT", [E, D], f32, kind="ExternalInput")
    bt_d = nc.dram_tensor("bt", [D, 1], f32, kind="ExternalInput")
    wup_d = nc.dram_tensor("wup", [D, 1], f32, kind="ExternalInput")
    wprod_d = nc.dram_tensor("wprod", [D, 1], f32, kind="ExternalInput")
    WcT_d = nc.dram_tensor("WcT", [D, C0], cdt, kind="ExternalInput")
    biascol_d = nc.dram_tensor("biascol", [128, 1], f32, kind="ExternalInput")
    Wd_d = [
        nc.dram_tensor(f"Wd{i}", [128, DCOLS[i][1]], cdt, kind="ExternalInput")
        for i in range(3)
    ]
    Tw_d = [
        nc.dram_tensor(f"Tw{i}", [128, TCOLS[i][1]], cdt, kind="ExternalInput")
        for i in range(3)
    ]
    Wfc_d = nc.dram_tensor("Wfc", [128, 2, 2, 256], f32, kind="ExternalInput")
    bfc_d = nc.dram_tensor("bfc", [1, 2], f32, kind="ExternalInput")
    zeros_d = nc.dram_tensor("zeros", [128, 132], cdt, kind="ExternalInput")
    out_d = nc.dram_tensor("out", [1, 2], f32, kind="ExternalOutput")

    with tile.TileContext(nc) as tc:
        from contextlib import ExitStack

        with ExitStack() as ctx:
            persist = ctx.enter_context(tc.tile_pool(name="persist", bufs=1))
            # persistent across block boundaries
            X2a = persist.tile([128, 66, 66], cdt, tag="X2a")
            X3a = persist.tile([128, 34, 34], cdt, tag="X3a")
            X3b = persist.tile([128, 34, 34], cdt, tag="X3b")

            def memset_borders(t, w):
                n = t.shape[0]
                nc.sync.dma_start(out=t[:, 0, :], in_=zeros_d[0:n, 0 : w + 2])
                nc.sync.dma_start(out=t[:, w + 1, :], in_=zeros_d[0:n, 0 : w + 2])
                nc.sync.dma_start(out=t[:, :, 0], in_=zeros_d[0:n, 0 : w + 2])
                nc.sync.dma_start(out=t[:, :, w + 1], in_=zeros_d[0:n, 0 : w + 2])

            # ====================================================== phase b1
            with ExitStack() as c1:
                p_b1 = c1.enter_context(tc.tile_pool(name="p_b1", bufs=1))
                Wsb1 = p_b1.tile([128, DCOLS[0][1]], cdt, tag="Wsb1")
                nc.sync.dma_start(out=Wsb1[:], in_=Wd_d[0][:])
                T1sb = p_b1.tile([128, TCOLS[0][1]], cdt, tag="T1sb")
                nc.sync.dma_start(out=T1sb[:], in_=Tw_d[0][:])
                WcT_sb = p_b1.tile([D, C0], cdt, tag="WcT")
                nc.sync.dma_start(out=WcT_sb[:], in_=WcT_d[:])
                biascol = p_b1.tile([128, 1], f32, tag="biascol")
                nc.sync.dma_start(out=biascol[:], in_=biascol_d[:])
                X1a = p_b1.tile([128, 130, 130], cdt, tag="X1a")
                peT = p_b1.tile([D, P], f32, tag="peT")
                heT = p_b1.tile([D, P], f32, tag="heT")
                memset_borders(X1a, 128)
                memset_borders(X2a, 64)
                memset_borders(X3a, 32)
                memset_borders(X3b, 32)

                # ---------------------------------------------- encoders
                with ExitStack() as ce:
                    enc = ce.enter_context(tc.tile_pool(name="enc", bufs=2))
                    encps = ce.enter_context(
                        tc.tile_pool(name="encps", bufs=2, space="PSUM")
                    )
                    consts = ce.enter_context(tc.tile_pool(name="consts", bufs=1))
                    ident = consts.tile([128, 128], f32, tag="ident")
                    make_identity(nc, ident[:])
                    WtT_sb = consts.tile([128, 6, 128], f32, tag="WtT")
                    nc.sync.dma_start(
                        out=WtT_sb[:], in_=WtT_d.rearrange("(k p) j -> p k j", p=128)
                    )
                    bt_sb = consts.tile([D, 1], f32, tag="bt")
                    nc.sync.dma_start(out=bt_sb[:], in_=bt_d[:])
                    wup_sb = consts.tile([D, 1], f32, tag="wup")
                    nc.sync.dma_start(out=wup_sb[:], in_=wup_d[:])
                    wprod_sb = consts.tile([D, 1], f32, tag="wprod")
                    nc.sync.dma_start(out=wprod_sb[:], in_=wprod_d[:])
                    ones1 = consts.tile([1, 128], f32, tag="ones1")
                    nc.vector.memset(ones1[:], 1.0)

                    for name, src_d, dst in (("p", pT_d, peT), ("h", hT_d, heT)):
                        with nc.named_scope(f"enc_{name}"):
                            XE = enc.tile([128, 6, 128], f32, tag="XE")
                            nc.sync.dma_start(
                                out=XE[:], in_=src_d.rearrange("(k p) j -> p k j", p=128)
                            )
                            psxT = encps.tile([128, 128], f32, tag="psxT")
                            for k in range(6):
                                nc.tensor.matmul(
                                    psxT[:],
                                    WtT_sb[:, k, :],
                                    XE[:, k, :],
                                    start=(k == 0),
                                    stop=(k == 5),
                                )
                            xT = enc.tile([128, 128], f32, tag="xT")
                            nc.scalar.activation(
                                xT[:], psxT[:], AF.Identity, bias=bt_sb[:]
                            )
                            # x (token-major) via PE transpose
                            psT = encps.tile([128, 128], f32, tag="psT")
                            nc.tensor.transpose(psT[:], xT[:], ident[:])
                            x_sb = enc.tile([128, 128], f32, tag="x_sb")
                            nc.vector.tensor_copy(x_sb[:], psT[:])
                            # u row
                            psu = encps.tile([1, 128], f32, tag="psu")
                            nc.tensor.matmul(
                                psu[0:1, :], wup_sb[:, 0:1], xT[:], start=True, stop=True
                            )
                            u_sb = enc.tile([1, 128], f32, tag="u_sb")
                            nc.vector.tensor_copy(u_sb[:], psu[0:1, :])
                            # A = (x*wprod) @ x.T + u[None, :]
                            xwT = enc.tile([128, 128], f32, tag="xwT")
                            nc.vector.tensor_scalar_mul(xwT[:], xT[:], wprod_sb[:, 0:1])
                            psA = encps.tile([128, 128], f32, tag="psA")
                            nc.tensor.matmul(psA[:], xwT[:], xT[:], start=True, stop=False)
                            nc.tensor.matmul(
                                psA[:], ones1[0:1, :], u_sb[0:1, :], start=False, stop=True
                            )
                            # softmax over free dim
                            negmax = enc.tile([128, 1], f32, tag="negmax")
                            nc.vector.tensor_reduce(
                                negmax[:], psA[:], axis=AX.X, op=ALU.max, negate=True
                            )
                            sAe = enc.tile([128, 128], f32, tag="sAe")
                            den = enc.tile([128, 1], f32, tag="den")
                            nc.scalar.activation(
                                sAe[:], psA[:], AF.Exp, bias=negmax[:], scale=1.0,
                                accum_out=den[:],
                            )
                            rden = enc.tile([128, 1], f32, tag="rden")
                            nc.vector.reciprocal(rden[:], den[:])
                            sAn = enc.tile([128, 128], f32, tag="sAn")
                            nc.vector.tensor_scalar_mul(sAn[:], sAe[:], rden[:])
                            psT2 = encps.tile([128, 128], f32, tag="psT")
                            nc.tensor.transpose(psT2[:], sAn[:], ident[:])
                            sAT = enc.tile([128, 128], f32, tag="sAT")
                            nc.vector.tensor_copy(sAT[:], psT2[:])
                            pspeT = encps.tile([128, 128], f32, tag="psA")
                            nc.tensor.matmul(
                                pspeT[:], x_sb[:], sAT[:], start=True, stop=True
                            )
                            nc.scalar.copy(dst[:], pspeT[:])

                # ------------------------------------- interaction + conv_in
                if PH >= 2:
                 with ExitStack() as ci:
                    pI = ci.enter_context(tc.tile_pool(name="pI", bufs=1))
                    cips = ci.enter_context(
                        tc.tile_pool(name="cips", bufs=2, space="PSUM")
                    )
                    stg0 = ci.enter_context(tc.tile_pool(name="stg0", bufs=4))
                    I = pI.tile([128, 128, 128], cdt, tag="I")  # [d, p, h]
                    with nc.named_scope("interaction"):
                        for p in range(P):
                            nc.vector.tensor_scalar_mul(
                                I[:, p, :], heT[:], peT[:, p : p + 1]
                            )
                    with nc.named_scope("conv_in"):
                        for g in range(16):
                            psl = [
                                cips.tile([128, 512], f32, tag=f"cip{cg}",
                                          name=f"cip{cg}")
                                for cg in range(2)
                            ]
                            for cg in range(2):
                                rhs = I[:, g * 8 + cg * 4 : g * 8 + cg * 4 + 4, :]
                                nc.tensor.matmul(
                                    psl[cg][64 * cg : 64 * cg + C0, :],
                                    WcT_sb[:],
                                    rhs,
                                    start=True,
                                    stop=True,
                                    tile_position=(0, 64 * cg),
                                )
                            st0 = stg0.tile([64, 2, 512], cdt, tag="st0")
                            for cg in range(2):
                                src = psl[cg][64 * cg : 64 * cg + C0, :]
                                dst = st0[0:C0, cg, :]
                                nc.scalar.activation(
                                    dst, src, AF.Identity, bias=biascol[0:C0, :]
                                )
                            nc.sync.dma_start(
                                out=X1a[0:C0, g * 8 + 1 : g * 8 + 9, 1:129],
                                in_=st0[0:C0].rearrange(
                                    "c g (r x) -> c (g r) x", x=128
                                ),
                            )

                # ------------------------------------------- dense block 1
                if PH >= 3:
                 with ExitStack() as cb:
                    px1b = cb.enter_context(tc.tile_pool(name="px1b", bufs=1))
                    X1b = px1b.tile([70, 130, 130], cdt, tag="X1b")
                    memset_borders(X1b, 128)
                    with tc.tile_pool(name="dps1", bufs=2, space="PSUM") as dps, \
                            tc.tile_pool(name="stg1", bufs=6) as stg:
                        _dense_block(nc, tc, 0, [X1a, X1b], Wsb1, dps, stg)
                    # transition 1 -> X2a
                    if PH >= 4:
                     with tc.tile_pool(name="tps1", bufs=2, space="PSUM") as tps, \
                            tc.tile_pool(name="tmp1", bufs=2) as tmpp:
                        _transition(nc, tc, 0, [X1a, X1b], T1sb, [X2a], tps, tmpp)

            # ====================================================== phase b2
            if PH >= 5:
             with ExitStack() as c2:
                p_b2 = c2.enter_context(tc.tile_pool(name="p_b2", bufs=1))
                Wsb2 = p_b2.tile([128, DCOLS[1][1]], cdt, tag="Wsb2")
                nc.sync.dma_start(out=Wsb2[:], in_=Wd_d[1][:])
                T2sb = p_b2.tile([128, TCOLS[1][1]], cdt, tag="T2sb")
                nc.sync.dma_start(out=T2sb[:], in_=Tw_d[1][:])
                X2b = p_b2.tile([128, 66, 66], cdt, tag="X2b")
                X2c = p_b2.tile([3, 66, 66], cdt, tag="X2c")
                memset_borders(X2b, 64)
                memset_borders(X2c, 64)
                with tc.tile_pool(name="dps2", bufs=2, space="PSUM") as dps, \
                        tc.tile_pool(name="stg2", bufs=6) as stg:
                    _dense_block(nc, tc, 1, [X2a, X2b, X2c], Wsb2, dps, stg)
                with tc.tile_pool(name="tps2", bufs=2, space="PSUM") as tps, \
                        tc.tile_pool(name="tmp2", bufs=2) as tmpp:
                    _transition(nc, tc, 1, [X2a, X2b, X2c], T2sb, [X3a, X3b], tps, tmpp)

            # ====================================================== phase b3
            if PH >= 6:
             with ExitStack() as c3:
                p_b3 = c3.enter_context(tc.tile_pool(name="p_b3", bufs=1))
                Wsb3 = p_b3.tile([128, DCOLS[2][1]], cdt, tag="Wsb3")
                nc.sync.dma_start(out=Wsb3[:], in_=Wd_d[2][:])
                T3sb = p_b3.tile([128, TCOLS[2][1]], cdt, tag="T3sb")
                nc.sync.dma_start(out=T3sb[:], in_=Tw_d[2][:])
                X3c = p_b3.tile([33, 34, 34], cdt, tag="X3c")
                memset_borders(X3c, 32)
                X4a = p_b3.tile([128, 16, 16], f32, tag="X4a")
                X4b = p_b3.tile([16, 16, 16], f32, tag="X4b")
                with tc.tile_pool(name="dps3", bufs=2, space="PSUM") as dps, \
                        tc.tile_pool(name="stg3", bufs=6) as stg:
                    _dense_block(nc, tc, 2, [X3a, X3b, X3c], Wsb3, dps, stg)
                if PH >= 7:
                    with tc.tile_pool(name="tps3", bufs=2, space="PSUM") as tps, \
                            tc.tile_pool(name="tmp3", bufs=2) as tmpp:
                        _transition(nc, tc, 2, [X3a, X3b, X3c], T3sb, [X4a, X4b], tps, tmpp)

                # --------------------------------------------------- FC head
                if PH >= 8:
                 with nc.named_scope("fc"):
                    Wfc_sb = p_b3.tile([128, 2, 2, 256], f32, tag="Wfc")
                    nc.sync.dma_start(out=Wfc_sb[:], in_=Wfc_d[:])
                    bfc_sb = p_b3.tile([1, 2], f32, tag="bfc")
                    nc.sync.dma_start(out=bfc_sb[:], in_=bfc_d[:])
                    P4 = p_b3.tile([128, 4], f32, tag="P4")
                    nc.vector.memset(P4[:], 0.0)
                    scr = p_b3.tile([128, 256], f32, tag="fscr")
                    for j in range(2):
                        for t, rows in ((0, 128), (1, 16)):
                            X4t = (X4a, X4b)[t]
                            nc.vector.tensor_mul(
                                scr[0:rows, :],
                                X4t[0:rows].rearrange("c r x -> c (r x)"),
                                Wfc_sb[0:rows, j, t, :],
                            )
                            nc.vector.reduce_sum(
                                P4[0:rows, 2 * j + t : 2 * j + t + 1],
                                scr[0:rows, :],
                                axis=mybir.AxisListType.X,
                            )
                    ones128 = p_b3.tile([128, 1], f32, tag="ones128")
                    nc.vector.memset(ones128[:], 1.0)
                    fps = c3.enter_context(tc.tile_pool(name="fps", bufs=1, space="PSUM"))
                    psf = fps.tile([1, 4], f32, tag="psf")
                    nc.tensor.matmul(
                        psf[0:1, :], ones128[:, 0:1], P4[:], start=True, stop=True
                    )
                    f4 = p_b3.tile([1, 4], f32, tag="f4")
                    nc.vector.tensor_copy(f4[:], psf[0:1, :])
                    out2 = p_b3.tile([1, 2], f32, tag="out2")
                    nc.vector.tensor_add(out2[:], f4[0:1, 0:4:2], f4[0:1, 1:4:2])
                    nc.vector.tensor_add(out2[:], out2[:], bfc_sb[:])
                    nc.sync.dma_start(out=out_d[:], in_=out2[:])

    if PH < 8:
        with tile.TileContext(nc) as tc2:
            with tc2.tile_pool(name="fallout", bufs=1) as fo:
                z = fo.tile([1, 2], f32, tag="zout")
                nc.vector.memset(z[:], 0.0)
                nc.sync.dma_start(out=out_d[:], in_=z[:])
    nc.finalize()
    return nc


def _dense_block(nc, tc, bl, Xt, Wsb, dps, stg):
    import concourse.mybir as mybir

    f32 = mybir.dt.float32
    cdt = mybir.dt.bfloat16 if CONV_DT == "bf16" else mybir.dt.float32
    AF = mybir.ActivationFunctionType
    spec = BLK[bl]
    W, caps, rpc = spec["W"], spec["caps"], spec["rpc"]
    N = rpc * W
    cols = DCOLS[bl][0]
    for l in range(NL):
        cin = spec["cin0"] + GR * l
        chunks = _chunks_of(cin, caps)
        outs = _pieces_of(cin, cin + GR, caps)
        n_mm = 9 * len(chunks)
        with nc.named_scope(f"b{bl + 1}_l{l}"):
            for g in range(spec["groups"]):
                psl = [
                    dps.tile([128, N], f32, tag=f"dps{cg}", name=f"dps{bl}_{cg}")
                    for cg in range(4)
                ]
                i = 0
                for ci, (ti, kc) in enumerate(chunks):
                    for ky in (-1, 0, 1):
                        for kx in (-1, 0, 1):
                            col = cols[(l, ci, ky, kx)]
                            for cg in range(4):
                                r0 = (g * 4 + cg) * rpc
                                rhs = Xt[ti][
                                    0:kc,
                                    r0 + 1 + ky : r0 + 1 + ky + rpc,
                                    1 + kx : 1 + kx + W,
                                ]
                                nc.tensor.matmul(
                                    psl[cg][32 * cg : 32 * cg + GR, :],
                                    Wsb[0:kc, col : col + GR],
                                    rhs,
                                    start=(i == 0),
                                    stop=(i == n_mm - 1),
                                    tile_position=(0, 32 * cg),
                                )
                            i += 1
                st = stg.tile([32, 4, N], cdt, tag="st")
                for cg in range(4):
                    src = psl[cg][32 * cg : 32 * cg + GR, :]
                    dst = st[0:GR, cg, :]
                    if cg != 1:
                        nc.scalar.activation(dst, src, AF.Relu)
                    else:
                        nc.vector.tensor_relu(dst, src)
                r0 = g * 4 * rpc
                dma_eng = nc.sync if g % 2 == 0 else nc.scalar
                for ti, lo, off, n in outs:
                    dma_eng.dma_start(
                        out=Xt[ti][lo : lo + n, r0 + 1 : r0 + 1 + 4 * rpc, 1 : 1 + W],
                        in_=st[off : off + n].rearrange(
                            "c g (r x) -> c (g r) x", x=W
                        ),
                    )


def _transition(nc, tc, bl, Xt, Tsb, Xn, tps, tmpp):
    import concourse.mybir as mybir

    f32 = mybir.dt.float32
    spec = TRS[bl]
    W = spec["W"]
    rpg = 512 // W
    ngr = W // rpg
    chunks = _chunks_of(spec["cin"], BLK[bl]["caps"])
    cols = TCOLS[bl][0]
    padded_out = bl < 2  # t3 output (X4) is unpadded
    with nc.named_scope(f"trans{bl + 1}"):
        for g in range(ngr):
            r0 = g * rpg
            psl = []
            for mi, (mlo, mn) in enumerate(spec["mts"]):
                psl.append(
                    tps.tile([128, 512], f32, tag=f"tps{mi}", name=f"tps{bl}_{mi}")
                )
            for ci, (ti, kc) in enumerate(chunks):
                rhs = Xt[ti][0:kc, r0 + 1 : r0 + 1 + rpg, 1 : 1 + W]
                for mi, (mlo, mn) in enumerate(spec["mts"]):
                    col = cols[(ci, mi)]
                    nc.tensor.matmul(
                        psl[mi][0:mn, :],
                        Tsb[0:kc, col : col + mn],
                        rhs,
                        start=(ci == 0),
                        stop=(ci == len(chunks) - 1),
                    )
            for mi, (mlo, mn) in enumerate(spec["mts"]):
                psv = psl[mi][0:mn, :].rearrange(
                    "c (r x two) -> c r x two", r=rpg, two=2
                )
                tmp = tmpp.tile([128, rpg, W // 2], f32, tag="pooltmp")
                nc.vector.tensor_reduce(
                    tmp[0:mn],
                    psv,
                    axis=mybir.AxisListType.X,
                    op=mybir.AluOpType.max,
                )
                ro = r0 // 2
                dst_t = Xn[mi]
                if padded_out:
                    dst = dst_t[0:mn, ro + 1 : ro + 1 + rpg // 2, 1 : 1 + W // 2]
                else:
                    dst = dst_t[0:mn, ro : ro + rpg // 2, :]
                nc.vector.tensor_max(
                    dst, tmp[0:mn, 0 : rpg : 2, :], tmp[0:mn, 1 : rpg : 2, :]
                )


# ----------------------------------------------------------------- host side
def _pack_weights(inputs):
    import ml_dtypes
    f = np.float32
    cnp = ml_dtypes.bfloat16 if CONV_DT == "bf16" else np.float32
    Wt = inputs["Wt"]  # (D, E)
    WtT = np.ascontiguousarray(Wt.T, dtype=f)  # (E, D)
    bt = inputs["bt"].reshape(D, 1).astype(f)
    w_attn = inputs["w_attn"]
    wup = w_attn[0:D].reshape(D, 1).astype(f)
    wprod = w_attn[2 * D : 3 * D].reshape(D, 1).astype(f)
    WcT = np.ascontiguousarray(inputs["Wc_in"][:, :, 0, 0].T, dtype=f)  # (D, C0)
    biascol = np.zeros((128, 1), f)
    biascol[0:C0, 0] = inputs["bc_in"]
    biascol[64 : 64 + C0, 0] = inputs["bc_in"]

    Wd = []
    for bl in range(3):
        cols, total = DCOLS[bl]
        ws = inputs[f"dense{bl + 1}_ws"]
        arr = np.zeros((128, total), f)
        spec = BLK[bl]
        for l in range(NL):
            w = np.asarray(ws[l])  # (GR, cin, 3, 3)
            cin = spec["cin0"] + GR * l
            for ci, (ti, kc) in enumerate(_chunks_of(cin, spec["caps"])):
                base = 128 * ti
                for iky, ky in enumerate((-1, 0, 1)):
                    for ikx, kx in enumerate((-1, 0, 1)):
                        c = cols[(l, ci, ky, kx)]
                        # lhsT[r, m] = w[m, base + r, iky, ikx]
                        arr[0:kc, c : c + GR] = w[:, base : base + kc, iky, ikx].T
        Wd.append(arr)

    Tw = []
    for bl in range(3):
        cols, total = TCOLS[bl]
        tw = np.asarray(inputs[f"trans{bl + 1}_w"])  # (nout, cin, 1, 1)
        arr = np.zeros((128, total), f)
        for ci, (ti, kc) in enumerate(_chunks_of(TRS[bl]["cin"], BLK[bl]["caps"])):
            base = 128 * ti
            for mi, (mlo, mn) in enumerate(TRS[bl]["mts"]):
                c = cols[(ci, mi)]
                arr[0:kc, c : c + mn] = tw[mlo : mlo + mn, base : base + kc, 0, 0].T
        Tw.append(arr)

    Wfc = np.asarray(inputs["Wfc"])  # (2, 36864)
    Wfc_arr = np.zeros((128, 2, 2, 256), f)
    wv = Wfc.reshape(2, 144, 256)
    for t, rows in ((0, 128), (1, 16)):
        Wfc_arr[0:rows, :, t, :] = np.transpose(wv[:, 128 * t : 128 * t + rows, :], (1, 0, 2))
    bfc = inputs["bfc"].reshape(1, 2).astype(f)

    return dict(
        WtT=WtT, bt=bt, wup=wup, wprod=wprod, WcT=WcT.astype(cnp),
        biascol=biascol,
        zeros=np.zeros((128, 132), cnp),
        Wd0=Wd[0].astype(cnp), Wd1=Wd[1].astype(cnp), Wd2=Wd[2].astype(cnp),
        Tw0=Tw[0].astype(cnp), Tw1=Tw[1].astype(cnp), Tw2=Tw[2].astype(cnp),
        Wfc=Wfc_arr, bfc=bfc,
    )


_NC_CACHE = {}


def kernel(**inputs):
    from concourse.bass_utils import run_bass_kernel_spmd

    if "nc" not in _NC_CACHE:
        _NC_CACHE["nc"] = _build_nc()
    nc = _NC_CACHE["nc"]

    shared = _pack_weights(inputs)
    p_emb = np.asarray(inputs["p_emb"], dtype=np.float32)
    h_emb = np.asarray(inputs["h_emb"], dtype=np.float32)
    in_maps = []
    for c in range(B):
        m = dict(shared)
        m["pT"] = np.ascontiguousarray(p_emb[c].T)
        m["hT"] = np.ascontiguousarray(h_emb[c].T)
        in_maps.append(m)

    res = run_bass_kernel_spmd(nc, in_maps, core_ids=list(range(B)))
    out = np.stack([r["out"].reshape(2) for r in res.results], axis=0)
    return out.astype(np.float32)


if __name__ == "__main__":
    nc = _build_nc()
    print("built ok")
